# revision 31
# baseline (speedup 1.0000x reference)
"""DigitCapsuleLayer (dynamic routing) Trainium2 Bass kernel.

Sharding: P-parallel — the 1152 primary capsules are split 144-per-core
across 8 cores; every core holds the full batch B=128 on SBUF partitions.

Per core (all heavy data bf16):
  phase 1 (TensorE): 18 chunks of 8 p's. lhsT = x chunk [(8p,8i)=64, 128b]
    (stationary, bf16 -> FWL), rhs = block-diagonal W [(8p,8i), 8p*160]
    -> u_hat[b, (p,od)] in one K=64 matmul pair per chunk (N=512+512+256),
    plus one dense-W accumulating matmul per chunk building s1 = sum_p u_hat.
    PSUM is evacuated to TWO bf16 SBUF copies of u_hat: u_pod [b,(p o d)]
    (ScalarE+DVE split): p-major for the b-update, and u_odp [b,(o d p)]
    (p-innermost) for the weighted sum — each layout keeps the innermost
    axis of every big DVE tensor_tensor op unit-stride so the bf16 2x mode
    engages.
  routing (DVE): muls are bf16 tensor_tensor (2x); segment reductions are
    contiguous bf16 halving-tree adds (2x) instead of 1x tensor_reduce.
  cross-core: AllReduce (gpsimd collective) of the [128,160] fp32 partial s
    for iters 1 and 2; iter 3's partial s is returned and reduced on host.
"""

import sys

sys.path.insert(0, "/opt/trn_rl_repo")

import numpy as np
import ml_dtypes

BF16 = ml_dtypes.bfloat16

B, P, IN_D, O, D = 128, 1152, 8, 10, 16
OD = O * D           # 160
NCORES = 8
PLOC = P // NCORES   # 144
EPS = 1e-8

CH = 8               # p's per phase-1 chunk
NCH = PLOC // CH     # 18
KCH = CH * IN_D      # 64 contraction rows per chunk
NBD = CH * OD        # 1280 block-diag output cols per chunk

import os as _os
ALT = not bool(_os.environ.get("CAPS_NO_ALT"))   # PE row-group alternation
NROW = 2 if ALT else 1
NCOL = NCH // NROW   # column blocks in the packed phase-1 layouts

_CACHE = {}


def _build():
    import os
    from concourse import bass, bacc, tile, mybir

    no_cc = bool(os.environ.get("CAPS_NO_CC"))
    debug = bool(os.environ.get("CAPS_DEBUG"))
    f32 = mybir.dt.float32
    bf = mybir.dt.bfloat16
    nc = bacc.Bacc("TRN2", target_bir_lowering=False, debug=False,
                   num_devices=1 if no_cc else NCORES)

    xT_d = nc.dram_tensor("xT", [NROW * KCH, NCOL * B], bf,
                          kind="ExternalInput")
    wBD_d = nc.dram_tensor("wBD", [NROW * KCH, NCOL * NBD], bf,
                           kind="ExternalInput")
    wDN_d = nc.dram_tensor("wDN", [NROW * KCH, NCOL * OD], bf,
                           kind="ExternalInput")
    out_d = nc.dram_tensor("sp3", [B, OD], f32, kind="ExternalOutput")
    if debug:
        dbg_d = {
            "d_upod": nc.dram_tensor("d_upod", [B, PLOC * OD], bf,
                                     kind="ExternalOutput"),
            "d_uodp": nc.dram_tensor("d_uodp", [B, PLOC * OD], bf,
                                     kind="ExternalOutput"),
            "d_s1": nc.dram_tensor("d_s1", [B, OD], f32,
                                   kind="ExternalOutput"),
            "d_br1": nc.dram_tensor("d_br1", [B, PLOC * O], f32,
                                    kind="ExternalOutput"),
            "d_ct2": nc.dram_tensor("d_ct2", [B, O * PLOC], bf,
                                    kind="ExternalOutput"),
            "d_s2": nc.dram_tensor("d_s2", [B, OD], f32,
                                   kind="ExternalOutput"),
        }

    AXX = None

    with tile.TileContext(nc) as tc:
        with (
            tc.tile_pool(name="persist", bufs=1) as pp,
            tc.tile_pool(name="dram", bufs=2, space="DRAM") as dp,
            tc.tile_pool(name="psum_ub", bufs=2, space="PSUM") as pub,
            tc.tile_pool(name="psum_s1", bufs=1, space="PSUM") as ps1,
        ):
            u_pod = pp.tile([B, PLOC * OD], bf)       # 45 KB/partition
            u_odp = pp.tile([B, PLOC * OD], bf)       # 45 KB/partition
            upod_f = u_pod[:]
            uodp_f = u_odp[:]
            upod4 = upod_f.rearrange("b (p o d) -> b p o d", p=PLOC, o=O, d=D)
            uodp4 = uodp_f.rearrange("b (o d p) -> b o d p", o=O, d=D, p=PLOC)

            # separate s1 accumulators per PE row-tile (T0/T8 must not
            # share a PSUM bank); dense W carries the 0.1 iter-1 scale
            s1a_ps = ps1.tile([B, OD], f32, padded_shape=[B, 512])
            s1b_ps = ps1.tile([B, OD], f32, padded_shape=[B, 512])

            # small persistent routing tiles needed across phase boundaries
            s_sb = pp.tile([B, OD], f32)
            s_full = pp.tile([B, OD], f32)

            def allreduce_start(src_ap, n, tag):
                if no_cc:
                    return src_ap
                cin = dp.tile([B, n], f32, tag=f"cin{tag}")
                cout = dp.tile([B, n], f32, tag=f"cout{tag}",
                               addr_space="Shared")
                nc.sync.dma_start(cin[:], src_ap)
                nc.gpsimd.collective_compute(
                    "AllReduce", mybir.AluOpType.add,
                    replica_groups=[list(range(NCORES))],
                    ins=[cin.opt()], outs=[cout.opt()],
                )
                return cout

            def allreduce_finish(cout, dst_ap):
                if no_cc:
                    nc.vector.tensor_copy(dst_ap, cout)
                    return
                nc.sync.dma_start(dst_ap, cout[:])

            def allreduce(src_ap, dst_ap, n, tag):
                allreduce_finish(allreduce_start(src_ap, n, tag), dst_ap)

            # ---------------- phase 1: u_hat + s1 ----------------
            # with ALT, chunks alternate PE row groups (partitions 0-63 /
            # 64-127) so each chunk's LDWEIGHTS overlaps the other group's
            # matmuls
            with tc.tile_pool(name="p1", bufs=1) as p1:
                xall = p1.tile([NROW * KCH, NCOL * B], bf)
                wdnall = p1.tile([NROW * KCH, NCOL * OD], bf)
                wbdall = p1.tile([NROW * KCH, NCOL * NBD], bf)
                nc.sync.dma_start(xall[:], xT_d[:])
                nc.sync.dma_start(wdnall[:], wDN_d[:])
                # wBD in thirds so chunk-0 matmuls start early
                third = (NCOL // 3) * NBD
                for t in range(3):
                    nc.sync.dma_start(wbdall[:, t * third:(t + 1) * third],
                                      wBD_d[:, t * third:(t + 1) * third])

                def rows(g):
                    r0 = KCH * (g % NROW)
                    return slice(r0, r0 + KCH)

                # s1 first: its AllReduce overlaps the rest of phase 1
                for g in range(NCH):
                    j = g // NROW
                    s1_ps = s1a_ps if g % NROW == 0 else s1b_ps
                    nc.tensor.matmul(
                        s1_ps[:], xall[:][rows(g), j * B:(j + 1) * B],
                        wdnall[:][rows(g), j * OD:(j + 1) * OD],
                        start=(g < NROW), stop=(g >= NCH - NROW),
                        skip_group_check=True,
                    )
                if ALT:
                    nc.vector.tensor_add(s_sb[:], s1a_ps[:], s1b_ps[:])
                else:
                    nc.vector.tensor_copy(s_sb[:], s1a_ps[:])
                if debug:
                    nc.sync.dma_start(dbg_d["d_s1"][:], s_sb[:])
                allreduce(s_sb[:], s_full[:], OD, "s1")

                for g in range(NCH):
                    j = g // NROW
                    xg = xall[:][rows(g), j * B:(j + 1) * B]
                    # pad to 3 full PSUM banks so each 512-col matmul
                    # output slice stays within one bank in both buffers
                    ub = pub.tile([B, NBD], f32, tag="ub",
                                  padded_shape=[B, 1536])
                    for n0, n1 in ((0, 512), (512, 1024), (1024, 1280)):
                        nc.tensor.matmul(
                            ub[:, n0:n1], xg,
                            wbdall[:][rows(g), j * NBD + n0:j * NBD + n1],
                            start=True, stop=True,
                        )
                    # evac: DVE -> p-major copy, ScalarE -> p-innermost copy
                    # (one p-major chunk goes to ScalarE for balance)
                    if g == 9:
                        nc.scalar.copy(upod_f[:, g * NBD:(g + 1) * NBD], ub[:])
                    else:
                        nc.vector.tensor_copy(
                            upod_f[:, g * NBD:(g + 1) * NBD], ub[:])
                    nc.scalar.copy(
                        uodp4[:, :, :, g * CH:(g + 1) * CH],
                        ub[:].rearrange("b (p o d) -> b o d p", p=CH, o=O, d=D))

            with tc.tile_pool(name="work", bufs=1) as wp:
                # ---------------- routing tiles ----------------
                tmp = wp.tile([B, PLOC * OD], bf)         # 45 KB/partition
                tmp_f = tmp[:]
                tmp4 = tmp_f.rearrange("b (p o d) -> b p o d", p=PLOC, o=O, d=D)
                tmp4o = tmp_f.rearrange("b (o d p) -> b o d p", o=O, d=D, p=PLOC)

                b_route = wp.tile([B, PLOC * O], f32)
                delta = wp.tile([B, PLOC * O], f32)
                e_t = wp.tile([B, PLOC * O], f32)
                e3 = e_t[:].rearrange("b (p o) -> b p o", p=PLOC, o=O)
                cT = wp.tile([B, O * PLOC], bf)
                cT3 = cT[:].rearrange("b (o p) -> b o p", o=O, p=PLOC)
                zs = wp.tile([B, PLOC], f32)
                rz = wp.tile([B, PLOC], f32)

                sq = wp.tile([B, OD], f32)
                v_bf = wp.tile([B, OD], bf)
                n2 = wp.tile([B, O], f32)
                rt = wp.tile([B, O], f32)
                a1 = wp.tile([B, O], f32)
                a2 = wp.tile([B, O], f32)
                den = wp.tile([B, O], f32)
                rec = wp.tile([B, O], f32)
                g_t = wp.tile([B, O], f32)

                AX = mybir.AxisListType.X

                def bcast(a, b_ap):
                    return bass.broadcast_tensor_aps(a, b_ap)

                OSL_ALL = slice(0, O)

                def squash(osl=OSL_ALL):
                    # v = (n2/(1+n2)) * s / (sqrt(n2)+eps), per (b, o)
                    c0, c1 = osl.start * D, osl.stop * D
                    no = osl.stop - osl.start
                    nc.vector.tensor_mul(sq[:, c0:c1], s_full[:, c0:c1],
                                         s_full[:, c0:c1])
                    nc.vector.reduce_sum(
                        n2[:, osl],
                        sq[:, c0:c1].rearrange("b (o d) -> b o d", o=no, d=D),
                        axis=AX)
                    nc.scalar.sqrt(rt[:, osl], n2[:, osl])
                    nc.vector.tensor_scalar_add(a1[:, osl], n2[:, osl], 1.0)
                    nc.vector.tensor_scalar_add(a2[:, osl], rt[:, osl], EPS)
                    nc.vector.tensor_mul(den[:, osl], a1[:, osl], a2[:, osl])
                    nc.vector.reciprocal(rec[:, osl], den[:, osl])
                    nc.vector.tensor_mul(g_t[:, osl], n2[:, osl], rec[:, osl])
                    sf3 = s_full[:, c0:c1].rearrange("b (o d) -> b o d",
                                                     o=no, d=D)
                    v3 = v_bf[:, c0:c1].rearrange("b (o d) -> b o d",
                                                  o=no, d=D)
                    ga, gb = bcast(sf3, g_t[:, osl].unsqueeze(-1))
                    nc.vector.tensor_mul(v3, ga, gb)

                def bupd(first, osl=OSL_ALL):
                    # delta[b,p,o] = sum_d u_pod[b,p,o,d] * v[b,o,d]
                    c0, c1 = osl.start * D, osl.stop * D
                    no = osl.stop - osl.start
                    va = v_bf[:, c0:c1].rearrange("b (o d) -> b o d",
                                                  o=no, d=D).unsqueeze(1)
                    ua, vb = bcast(upod4[:, :, osl, :], va)
                    t4 = tmp4[:, :, osl, :]
                    nc.vector.tensor_mul(t4, ua, vb)
                    # halving tree over d (innermost, contiguous bf16 runs)
                    nc.vector.tensor_add(t4[:, :, :, 0:8], t4[:, :, :, 0:8],
                                         t4[:, :, :, 8:16])
                    nc.vector.tensor_add(t4[:, :, :, 0:4], t4[:, :, :, 0:4],
                                         t4[:, :, :, 4:8])
                    nc.vector.tensor_add(t4[:, :, :, 0:2], t4[:, :, :, 0:2],
                                         t4[:, :, :, 2:4])
                    dst3 = (b_route if first else delta)[:].rearrange(
                        "b (p o) -> b p o", p=PLOC, o=O)
                    nc.vector.tensor_add(
                        dst3[:, :, osl].unsqueeze(-1),
                        t4[:, :, :, 0:1], t4[:, :, :, 1:2])
                    if not first:
                        br3 = b_route[:].rearrange("b (p o) -> b p o",
                                                   p=PLOC, o=O)
                        de3 = delta[:].rearrange("b (p o) -> b p o",
                                                 p=PLOC, o=O)
                        nc.vector.tensor_add(br3[:, :, osl], br3[:, :, osl],
                                             de3[:, :, osl])

                def softmax():
                    # cT[b,o,p] = softmax_o(b_route)[b,p,o], bf16 o-major
                    nc.scalar.activation(e_t[:], b_route[:],
                                         mybir.ActivationFunctionType.Exp)
                    nc.vector.reduce_sum(zs[:], e3, axis=AX)
                    nc.vector.reciprocal(rz[:], zs[:])
                    ea = e3.transpose([0, 2, 1])      # [b, o, p] view
                    ra = rz[:].unsqueeze(1)           # [b, 1, p]
                    ea2, rb = bcast(ea, ra)
                    nc.vector.tensor_mul(cT3, ea2, rb)

                def weighted_s(osl=OSL_ALL):
                    # tmp[b,o,d,p] = cT[b,o,p] * u_odp[b,o,d,p]; tree over p
                    c0, c1 = osl.start * D, osl.stop * D
                    ca = cT3[:, osl, :].unsqueeze(2)  # [b, o, 1, p]
                    ua, cb = bcast(uodp4[:, osl, :, :], ca)
                    nc.vector.tensor_mul(tmp4o[:, osl, :, :], ua, cb)
                    # halving tree over p (innermost runs): 144->...->9
                    t3 = tmp_f.rearrange("b (od p) -> b od p", od=OD,
                                         p=PLOC)[:, c0:c1, :]
                    for h in (72, 36, 18, 9):
                        nc.vector.tensor_add(t3[:, :, 0:h], t3[:, :, 0:h],
                                             t3[:, :, h:2 * h])
                    # 9 = 4+4+carry(col 8)
                    nc.vector.tensor_add(t3[:, :, 0:4], t3[:, :, 0:4],
                                         t3[:, :, 4:8])
                    nc.vector.tensor_add(t3[:, :, 0:2], t3[:, :, 0:2],
                                         t3[:, :, 2:4])
                    nc.vector.tensor_add(t3[:, :, 0:1], t3[:, :, 0:1],
                                         t3[:, :, 1:2])
                    nc.vector.tensor_add(t3[:, :, 0:1], t3[:, :, 0:1],
                                         t3[:, :, 8:9])
                    nc.vector.tensor_copy(s_sb[:, c0:c1].unsqueeze(2),
                                          t3[:, :, 0:1])

                # ---------------- routing ----------------
                # iter 1: c uniform = 1/10 (s1 scale + AllReduce emitted in
                # phase 1 so the collective overlaps the block matmuls)
                if debug:
                    nc.sync.dma_start(dbg_d["d_upod"][:], upod_f)
                    nc.sync.dma_start(dbg_d["d_uodp"][:], uodp_f)
                squash()
                bupd(first=True)
                if debug:
                    nc.sync.dma_start(dbg_d["d_br1"][:], b_route[:])

                # iter 2: o-halved so each AllReduce overlaps compute
                softmax()
                H0, H1 = slice(0, O // 2), slice(O // 2, O)
                weighted_s(H0)
                allreduce(s_sb[:, 0:80], s_full[:, 0:80], 80, "a")
                weighted_s(H1)
                if debug:
                    nc.sync.dma_start(dbg_d["d_ct2"][:], cT[:])
                    nc.sync.dma_start(dbg_d["d_s2"][:], s_sb[:])
                arb = allreduce_start(s_sb[:, 80:160], 80, "b")
                squash(H0)
                bupd(False, H0)
                allreduce_finish(arb, s_full[:, 80:160])
                squash(H1)
                bupd(False, H1)

                # iter 3: partial s only; reduce + squash on host
                softmax()
                weighted_s()
                nc.sync.dma_start(out_d[:], s_sb[:])

    nc.compile()
    return nc


def _get_nc():
    if "nc" not in _CACHE:
        _CACHE["nc"] = _build()
    return _CACHE["nc"]


def _pairify(a):
    # [NCH, KCH, N] -> [NROW*KCH, NCOL*N]: chunk g at row block g%NROW,
    # column block g//NROW
    NCHh, K, N = a.shape
    out = np.zeros((NROW * K, NCOL * N), dtype=a.dtype)
    for g in range(NCHh):
        out[K * (g % NROW):K * (g % NROW) + K,
            (g // NROW) * N:(g // NROW + 1) * N] = a[g]
    return out


def _prep_core(x, W, c):
    sl = slice(c * PLOC, (c + 1) * PLOC)
    xs = x[:, sl, :]                                   # [B, 144, 8]
    Wod = W[0, sl].reshape(PLOC, OD, IN_D)             # [144, 160, 8]
    # lhsT chunks: [NCH, (CH p, 8 i) = KCH, B]
    xT2 = xs.transpose(1, 2, 0).reshape(NCH, KCH, B).astype(BF16)
    # dense W stack: [NCH, KCH, OD]; carries the 0.1 iter-1 c-scale
    Wt = Wod.transpose(0, 2, 1)                        # [144, 8, 160]
    wDN = (0.1 * Wt.reshape(NCH, KCH, OD)).astype(BF16)
    # block-diagonal W: [NCH, KCH, CH*OD]
    wBD = np.zeros((NCH, KCH, NBD), dtype=BF16)
    Wc = Wt.reshape(NCH, CH, IN_D, OD)
    for ps in range(CH):
        wBD[:, ps * IN_D:(ps + 1) * IN_D, ps * OD:(ps + 1) * OD] = Wc[:, ps]
    return {"xT": _pairify(xT2), "wBD": _pairify(wBD), "wDN": _pairify(wDN)}


def kernel(x: np.ndarray, W: np.ndarray) -> np.ndarray:
    import os
    from concourse.bass_utils import run_bass_kernel_spmd

    nc = _get_nc()
    trace = bool(os.environ.get("CAPS_TRACE"))
    x = np.ascontiguousarray(x, dtype=np.float32)
    W = np.ascontiguousarray(W, dtype=np.float32)

    in_maps = [_prep_core(x, W, c) for c in range(NCORES)]

    res = run_bass_kernel_spmd(nc, in_maps, list(range(NCORES)),
                               trace=trace,
                               tmpdir=os.environ.get("CAPS_TRACE_DIR"))
    if trace:
        print(f"HW exec time: {res.exec_time_ns} ns")
        _CACHE["last_result"] = res
    s = np.zeros((B, OD), dtype=np.float32)
    for c in range(NCORES):
        s += res.results[c]["sp3"]

    s = s.reshape(B, O, D)
    n2 = np.sum(s * s, axis=-1, keepdims=True, dtype=np.float32)
    norm = np.sqrt(n2)
    v = (n2 / (1.0 + n2)) * s / (norm + EPS)
    return v.astype(np.float32)


# revision 33
# speedup vs baseline: 1.1270x; 1.1270x over previous
"""DigitCapsuleLayer (dynamic routing) Trainium2 Bass kernel.

Sharding: P-parallel — the 1152 primary capsules are split 144-per-core
across 8 cores; every core holds the full batch B=128 on SBUF partitions.

Per core (all heavy data bf16):
  phase 1 (TensorE): 18 chunks of 8 p's. lhsT = x chunk [(8p,8i)=64, 128b]
    (stationary, bf16 -> FWL), rhs = block-diagonal W [(8p,8i), 8p*160]
    -> u_hat[b, (p,od)] in one K=64 matmul pair per chunk (N=512+512+256),
    plus one dense-W accumulating matmul per chunk building s1 = sum_p u_hat.
    PSUM is evacuated to TWO bf16 SBUF copies of u_hat: u_pod [b,(p o d)]
    (ScalarE+DVE split): p-major for the b-update, and u_odp [b,(o d p)]
    (p-innermost) for the weighted sum — each layout keeps the innermost
    axis of every big DVE tensor_tensor op unit-stride so the bf16 2x mode
    engages.
  routing (DVE): muls are bf16 tensor_tensor (2x); segment reductions are
    contiguous bf16 halving-tree adds (2x) instead of 1x tensor_reduce.
  cross-core: AllReduce (gpsimd collective) of the [128,160] fp32 partial s
    for iters 1 and 2; iter 3's partial s is returned and reduced on host.
"""

import sys

sys.path.insert(0, "/opt/trn_rl_repo")

import numpy as np
import ml_dtypes

BF16 = ml_dtypes.bfloat16

B, P, IN_D, O, D = 128, 1152, 8, 10, 16
OD = O * D           # 160
NCORES = 8
PLOC = P // NCORES   # 144
EPS = 1e-8

CH = 8               # p's per phase-1 chunk
NCH = PLOC // CH     # 18
KCH = CH * IN_D      # 64 contraction rows per chunk
NBD = CH * OD        # 1280 block-diag output cols per chunk

import os as _os
ALT = not bool(_os.environ.get("CAPS_NO_ALT"))   # PE row-group alternation
NROW = 2 if ALT else 1
NCOL = NCH // NROW   # column blocks in the packed phase-1 layouts

_CACHE = {}


def _build():
    import os
    from concourse import bass, bacc, tile, mybir

    no_cc = bool(os.environ.get("CAPS_NO_CC"))
    debug = bool(os.environ.get("CAPS_DEBUG"))
    f32 = mybir.dt.float32
    bf = mybir.dt.bfloat16
    nc = bacc.Bacc("TRN2", target_bir_lowering=False, debug=False,
                   num_devices=1 if no_cc else NCORES)

    xT_d = nc.dram_tensor("xT", [NROW * KCH, NCOL * B], bf,
                          kind="ExternalInput")
    wBD_d = nc.dram_tensor("wBD", [NROW * KCH, NCOL * NBD], bf,
                           kind="ExternalInput")
    wDN_d = nc.dram_tensor("wDN", [NROW * KCH, NCOL * OD], bf,
                           kind="ExternalInput")
    out_d = nc.dram_tensor("sp3", [B, OD], f32, kind="ExternalOutput")
    if debug:
        dbg_d = {
            "d_upod": nc.dram_tensor("d_upod", [B, PLOC * OD], bf,
                                     kind="ExternalOutput"),
            "d_uodp": nc.dram_tensor("d_uodp", [B, PLOC * OD], bf,
                                     kind="ExternalOutput"),
            "d_s1": nc.dram_tensor("d_s1", [B, OD], f32,
                                   kind="ExternalOutput"),
            "d_br1": nc.dram_tensor("d_br1", [B, PLOC * O], f32,
                                    kind="ExternalOutput"),
            "d_ct2": nc.dram_tensor("d_ct2", [B, O * PLOC], bf,
                                    kind="ExternalOutput"),
            "d_s2": nc.dram_tensor("d_s2", [B, OD], f32,
                                   kind="ExternalOutput"),
        }

    AXX = None

    with tile.TileContext(nc) as tc:
        with (
            tc.tile_pool(name="persist", bufs=1) as pp,
            tc.tile_pool(name="dram", bufs=2, space="DRAM") as dp,
            tc.tile_pool(name="psum_ub", bufs=2, space="PSUM") as pub,
            tc.tile_pool(name="psum_s1", bufs=1, space="PSUM") as ps1,
        ):
            u_pod = pp.tile([B, PLOC * OD], bf)       # 45 KB/partition
            u_odp = pp.tile([B, PLOC * OD], bf)       # 45 KB/partition
            upod_f = u_pod[:]
            uodp_f = u_odp[:]
            upod4 = upod_f.rearrange("b (p o d) -> b p o d", p=PLOC, o=O, d=D)
            uodp4 = uodp_f.rearrange("b (o d p) -> b o d p", o=O, d=D, p=PLOC)

            # separate s1 accumulators per PE row-tile (T0/T8 must not
            # share a PSUM bank); dense W carries the 0.1 iter-1 scale
            s1a_ps = ps1.tile([B, OD], f32, padded_shape=[B, 512])
            s1b_ps = ps1.tile([B, OD], f32, padded_shape=[B, 512])

            # small persistent routing tiles needed across phase boundaries
            s_sb = pp.tile([B, OD], f32)
            s_full = pp.tile([B, OD], f32)

            def allreduce_start(src_ap, n, tag):
                if no_cc:
                    return src_ap
                cin = dp.tile([B, n], f32, tag=f"cin{tag}")
                cout = dp.tile([B, n], f32, tag=f"cout{tag}",
                               addr_space="Shared")
                nc.sync.dma_start(cin[:], src_ap)
                nc.gpsimd.collective_compute(
                    "AllReduce", mybir.AluOpType.add,
                    replica_groups=[list(range(NCORES))],
                    ins=[cin.opt()], outs=[cout.opt()],
                )
                return cout

            def allreduce_finish(cout, dst_ap):
                if no_cc:
                    nc.vector.tensor_copy(dst_ap, cout)
                    return
                nc.sync.dma_start(dst_ap, cout[:])

            def allreduce(src_ap, dst_ap, n, tag):
                allreduce_finish(allreduce_start(src_ap, n, tag), dst_ap)

            # ---------------- phase 1: u_hat + s1 ----------------
            # with ALT, chunks alternate PE row groups (partitions 0-63 /
            # 64-127) so each chunk's LDWEIGHTS overlaps the other group's
            # matmuls
            with tc.tile_pool(name="p1", bufs=1) as p1:
                xall = p1.tile([NROW * KCH, NCOL * B], bf)
                wdnall = p1.tile([NROW * KCH, NCOL * OD], bf)
                wbdall = p1.tile([NROW * KCH, NCOL * NBD], bf)
                nc.sync.dma_start(xall[:], xT_d[:])
                nc.sync.dma_start(wdnall[:], wDN_d[:])
                # wBD in thirds so chunk-0 matmuls start early
                third = (NCOL // 3) * NBD
                for t in range(3):
                    nc.sync.dma_start(wbdall[:, t * third:(t + 1) * third],
                                      wBD_d[:, t * third:(t + 1) * third])

                def rows(g):
                    r0 = KCH * (g % NROW)
                    return slice(r0, r0 + KCH)

                # s1 first: its AllReduce overlaps the rest of phase 1
                for g in range(NCH):
                    j = g // NROW
                    s1_ps = s1a_ps if g % NROW == 0 else s1b_ps
                    nc.tensor.matmul(
                        s1_ps[:], xall[:][rows(g), j * B:(j + 1) * B],
                        wdnall[:][rows(g), j * OD:(j + 1) * OD],
                        start=(g < NROW), stop=(g >= NCH - NROW),
                        skip_group_check=True,
                    )
                if ALT:
                    # DVE may read only one PSUM operand; stage b via SBUF
                    nc.scalar.copy(s_sb[:], s1b_ps[:])
                    nc.vector.tensor_add(s_sb[:], s_sb[:], s1a_ps[:])
                else:
                    nc.vector.tensor_copy(s_sb[:], s1a_ps[:])
                if debug:
                    nc.sync.dma_start(dbg_d["d_s1"][:], s_sb[:])
                allreduce(s_sb[:], s_full[:], OD, "s1")

                for g in range(NCH):
                    j = g // NROW
                    xg = xall[:][rows(g), j * B:(j + 1) * B]
                    # pad to 3 full PSUM banks so each 512-col matmul
                    # output slice stays within one bank in both buffers
                    ub = pub.tile([B, NBD], f32, tag="ub",
                                  padded_shape=[B, 1536])
                    for n0, n1 in ((0, 512), (512, 1024), (1024, 1280)):
                        nc.tensor.matmul(
                            ub[:, n0:n1], xg,
                            wbdall[:][rows(g), j * NBD + n0:j * NBD + n1],
                            start=True, stop=True,
                        )
                    # evac: DVE -> p-major copy, ScalarE -> p-innermost copy
                    # (one p-major chunk goes to ScalarE for balance)
                    if g == 9:
                        nc.scalar.copy(upod_f[:, g * NBD:(g + 1) * NBD], ub[:])
                    else:
                        nc.vector.tensor_copy(
                            upod_f[:, g * NBD:(g + 1) * NBD], ub[:])
                    nc.scalar.copy(
                        uodp4[:, :, :, g * CH:(g + 1) * CH],
                        ub[:].rearrange("b (p o d) -> b o d p", p=CH, o=O, d=D))

            with tc.tile_pool(name="work", bufs=1) as wp:
                # ---------------- routing tiles ----------------
                tmp = wp.tile([B, PLOC * OD], bf)         # 45 KB/partition
                tmp_f = tmp[:]
                tmp4 = tmp_f.rearrange("b (p o d) -> b p o d", p=PLOC, o=O, d=D)
                tmp4o = tmp_f.rearrange("b (o d p) -> b o d p", o=O, d=D, p=PLOC)

                b_route = wp.tile([B, PLOC * O], f32)
                delta = wp.tile([B, PLOC * O], f32)
                e_t = wp.tile([B, PLOC * O], f32)
                e3 = e_t[:].rearrange("b (p o) -> b p o", p=PLOC, o=O)
                cT = wp.tile([B, O * PLOC], bf)
                cT3 = cT[:].rearrange("b (o p) -> b o p", o=O, p=PLOC)
                zs = wp.tile([B, PLOC], f32)
                rz = wp.tile([B, PLOC], f32)

                sq = wp.tile([B, OD], f32)
                v_bf = wp.tile([B, OD], bf)
                n2 = wp.tile([B, O], f32)
                rt = wp.tile([B, O], f32)
                a1 = wp.tile([B, O], f32)
                a2 = wp.tile([B, O], f32)
                den = wp.tile([B, O], f32)
                rec = wp.tile([B, O], f32)
                g_t = wp.tile([B, O], f32)

                AX = mybir.AxisListType.X

                def bcast(a, b_ap):
                    return bass.broadcast_tensor_aps(a, b_ap)

                OSL_ALL = slice(0, O)

                def squash(osl=OSL_ALL):
                    # v = (n2/(1+n2)) * s / (sqrt(n2)+eps), per (b, o)
                    c0, c1 = osl.start * D, osl.stop * D
                    no = osl.stop - osl.start
                    nc.vector.tensor_mul(sq[:, c0:c1], s_full[:, c0:c1],
                                         s_full[:, c0:c1])
                    nc.vector.reduce_sum(
                        n2[:, osl],
                        sq[:, c0:c1].rearrange("b (o d) -> b o d", o=no, d=D),
                        axis=AX)
                    nc.scalar.sqrt(rt[:, osl], n2[:, osl])
                    nc.vector.tensor_scalar_add(a1[:, osl], n2[:, osl], 1.0)
                    nc.vector.tensor_scalar_add(a2[:, osl], rt[:, osl], EPS)
                    nc.vector.tensor_mul(den[:, osl], a1[:, osl], a2[:, osl])
                    nc.vector.reciprocal(rec[:, osl], den[:, osl])
                    nc.vector.tensor_mul(g_t[:, osl], n2[:, osl], rec[:, osl])
                    sf3 = s_full[:, c0:c1].rearrange("b (o d) -> b o d",
                                                     o=no, d=D)
                    v3 = v_bf[:, c0:c1].rearrange("b (o d) -> b o d",
                                                  o=no, d=D)
                    ga, gb = bcast(sf3, g_t[:, osl].unsqueeze(-1))
                    nc.vector.tensor_mul(v3, ga, gb)

                def bupd(first, osl=OSL_ALL):
                    # delta[b,p,o] = sum_d u_pod[b,p,o,d] * v[b,o,d]
                    c0, c1 = osl.start * D, osl.stop * D
                    no = osl.stop - osl.start
                    va = v_bf[:, c0:c1].rearrange("b (o d) -> b o d",
                                                  o=no, d=D).unsqueeze(1)
                    ua, vb = bcast(upod4[:, :, osl, :], va)
                    t4 = tmp4[:, :, osl, :]
                    nc.vector.tensor_mul(t4, ua, vb)
                    # halving tree over d (innermost, contiguous bf16 runs)
                    nc.vector.tensor_add(t4[:, :, :, 0:8], t4[:, :, :, 0:8],
                                         t4[:, :, :, 8:16])
                    nc.vector.tensor_add(t4[:, :, :, 0:4], t4[:, :, :, 0:4],
                                         t4[:, :, :, 4:8])
                    nc.vector.tensor_add(t4[:, :, :, 0:2], t4[:, :, :, 0:2],
                                         t4[:, :, :, 2:4])
                    dst3 = (b_route if first else delta)[:].rearrange(
                        "b (p o) -> b p o", p=PLOC, o=O)
                    nc.vector.tensor_add(
                        dst3[:, :, osl].unsqueeze(-1),
                        t4[:, :, :, 0:1], t4[:, :, :, 1:2])
                    if not first:
                        br3 = b_route[:].rearrange("b (p o) -> b p o",
                                                   p=PLOC, o=O)
                        de3 = delta[:].rearrange("b (p o) -> b p o",
                                                 p=PLOC, o=O)
                        nc.vector.tensor_add(br3[:, :, osl], br3[:, :, osl],
                                             de3[:, :, osl])

                def softmax():
                    # cT[b,o,p] = softmax_o(b_route)[b,p,o], bf16 o-major
                    nc.scalar.activation(e_t[:], b_route[:],
                                         mybir.ActivationFunctionType.Exp)
                    nc.vector.reduce_sum(zs[:], e3, axis=AX)
                    nc.vector.reciprocal(rz[:], zs[:])
                    ea = e3.transpose([0, 2, 1])      # [b, o, p] view
                    ra = rz[:].unsqueeze(1)           # [b, 1, p]
                    ea2, rb = bcast(ea, ra)
                    nc.vector.tensor_mul(cT3, ea2, rb)

                def weighted_s(osl=OSL_ALL):
                    # tmp[b,o,d,p] = cT[b,o,p] * u_odp[b,o,d,p]; tree over p
                    c0, c1 = osl.start * D, osl.stop * D
                    ca = cT3[:, osl, :].unsqueeze(2)  # [b, o, 1, p]
                    ua, cb = bcast(uodp4[:, osl, :, :], ca)
                    nc.vector.tensor_mul(tmp4o[:, osl, :, :], ua, cb)
                    # halving tree over p (innermost runs): 144->...->9
                    t3 = tmp_f.rearrange("b (od p) -> b od p", od=OD,
                                         p=PLOC)[:, c0:c1, :]
                    for h in (72, 36, 18, 9):
                        nc.vector.tensor_add(t3[:, :, 0:h], t3[:, :, 0:h],
                                             t3[:, :, h:2 * h])
                    # 9 = 4+4+carry(col 8)
                    nc.vector.tensor_add(t3[:, :, 0:4], t3[:, :, 0:4],
                                         t3[:, :, 4:8])
                    nc.vector.tensor_add(t3[:, :, 0:2], t3[:, :, 0:2],
                                         t3[:, :, 2:4])
                    nc.vector.tensor_add(t3[:, :, 0:1], t3[:, :, 0:1],
                                         t3[:, :, 1:2])
                    nc.vector.tensor_add(t3[:, :, 0:1], t3[:, :, 0:1],
                                         t3[:, :, 8:9])
                    nc.vector.tensor_copy(s_sb[:, c0:c1].unsqueeze(2),
                                          t3[:, :, 0:1])

                # ---------------- routing ----------------
                # iter 1: c uniform = 1/10 (s1 scale + AllReduce emitted in
                # phase 1 so the collective overlaps the block matmuls)
                if debug:
                    nc.sync.dma_start(dbg_d["d_upod"][:], upod_f)
                    nc.sync.dma_start(dbg_d["d_uodp"][:], uodp_f)
                squash()
                bupd(first=True)
                if debug:
                    nc.sync.dma_start(dbg_d["d_br1"][:], b_route[:])

                # iter 2
                softmax()
                weighted_s()
                if debug:
                    nc.sync.dma_start(dbg_d["d_ct2"][:], cT[:])
                    nc.sync.dma_start(dbg_d["d_s2"][:], s_sb[:])
                allreduce(s_sb[:], s_full[:], OD, "it2")
                squash()
                bupd(False)

                # iter 3: partial s only; reduce + squash on host
                softmax()
                weighted_s()
                nc.sync.dma_start(out_d[:], s_sb[:])

    nc.compile()
    return nc


def _get_nc():
    if "nc" not in _CACHE:
        _CACHE["nc"] = _build()
    return _CACHE["nc"]


def _pairify(a):
    # [NCH, KCH, N] -> [NROW*KCH, NCOL*N]: chunk g at row block g%NROW,
    # column block g//NROW
    NCHh, K, N = a.shape
    out = np.zeros((NROW * K, NCOL * N), dtype=a.dtype)
    for g in range(NCHh):
        out[K * (g % NROW):K * (g % NROW) + K,
            (g // NROW) * N:(g // NROW + 1) * N] = a[g]
    return out


def _prep_core(x, W, c):
    sl = slice(c * PLOC, (c + 1) * PLOC)
    xs = x[:, sl, :]                                   # [B, 144, 8]
    Wod = W[0, sl].reshape(PLOC, OD, IN_D)             # [144, 160, 8]
    # lhsT chunks: [NCH, (CH p, 8 i) = KCH, B]
    xT2 = xs.transpose(1, 2, 0).reshape(NCH, KCH, B).astype(BF16)
    # dense W stack: [NCH, KCH, OD]; carries the 0.1 iter-1 c-scale
    Wt = Wod.transpose(0, 2, 1)                        # [144, 8, 160]
    wDN = (0.1 * Wt.reshape(NCH, KCH, OD)).astype(BF16)
    # block-diagonal W: [NCH, KCH, CH*OD]
    wBD = np.zeros((NCH, KCH, NBD), dtype=BF16)
    Wc = Wt.reshape(NCH, CH, IN_D, OD)
    for ps in range(CH):
        wBD[:, ps * IN_D:(ps + 1) * IN_D, ps * OD:(ps + 1) * OD] = Wc[:, ps]
    return {"xT": _pairify(xT2), "wBD": _pairify(wBD), "wDN": _pairify(wDN)}


def kernel(x: np.ndarray, W: np.ndarray) -> np.ndarray:
    import os
    from concourse.bass_utils import run_bass_kernel_spmd

    nc = _get_nc()
    trace = bool(os.environ.get("CAPS_TRACE"))
    x = np.ascontiguousarray(x, dtype=np.float32)
    W = np.ascontiguousarray(W, dtype=np.float32)

    in_maps = [_prep_core(x, W, c) for c in range(NCORES)]

    res = run_bass_kernel_spmd(nc, in_maps, list(range(NCORES)),
                               trace=trace,
                               tmpdir=os.environ.get("CAPS_TRACE_DIR"))
    if trace:
        print(f"HW exec time: {res.exec_time_ns} ns")
        _CACHE["last_result"] = res
    s = np.zeros((B, OD), dtype=np.float32)
    for c in range(NCORES):
        s += res.results[c]["sp3"]

    s = s.reshape(B, O, D)
    n2 = np.sum(s * s, axis=-1, keepdims=True, dtype=np.float32)
    norm = np.sqrt(n2)
    v = (n2 / (1.0 + n2)) * s / (norm + EPS)
    return v.astype(np.float32)
